# revision 11
# baseline (speedup 1.0000x reference)
"""Bass/Trainium2 kernel for nn_DenoiserBlock (bidirectional mLSTM block).

Sharding: 8 cores = 4 batches x 2 directions (fwd / reversed). Each core runs
an identical SPMD program computing its direction's full contribution
y = (mlstm(x) + skip*conv(x)) * swish(z) @ W_down  for one batch element
(reverse cores see a time-flipped input). Host sums  out[b] = x[b] + y_f +
flip(y_r).

Key algebraic restructuring of the stabilized mLSTM:
  logD[i,j] = ci_i - ci_j + ipre_j  (ci = cumsum(logsigmoid(f_pre)))
  maxD_i = ci_i + M_i with M_i = prefixmax_j(g_j), g_j = ipre_j - ci_j
  => D[i,j]/exp(maxD_i) = exp(g_j) * exp(-M_i)   (separable, no S^2 exp)
so the decay is applied as a per-partition scalar multiply on the k^T q
tile, and all stabilization quantities are O(S) per head.
"""

import sys
import numpy as np

for _p in ("/opt/trn_rl_repo", "/root/.axon_site/_ro/trn_rl_repo"):
    if _p not in sys.path:
        sys.path.insert(0, _p)

import concourse.bass as bass
import concourse.tile as tile
from concourse import bacc, mybir
from concourse.masks import make_identity

AF = mybir.ActivationFunctionType
ALU = mybir.AluOpType
F32 = mybir.dt.float32
F32R = mybir.dt.float32r

B, S, E = 4, 1024, 512
INNER, NH, KS, BS = 1024, 4, 4, 4
DH = INNER // NH          # 256
NB = INNER // BS          # 256
P = 128
NCH = INNER // P          # 8 channel chunks of INNER
NST = S // P              # 8 sequence blocks
NEC = E // P              # 4 chunks of E
NCORES = 8

# matmul precision knobs: set to True to run that matmul group in float32r
# (full-rate PE) instead of float32 (quarter-rate).
R_UP = False     # up-projection
R_DOWN = False   # down-projection
R_ATT = False    # q k^T and (qk)^T v matmuls
R_PROJ = False   # headwise q/k/v projections + gate projections


def _r(ap, enable):
    return ap.bitcast(F32R) if enable else ap


# ----------------------------------------------------------------------------
# host-side weight folding
# ----------------------------------------------------------------------------

def _block_diag(w):
    """(NB, BS, BS) headwise blocks -> (INNER, INNER) [c_in, c_out]."""
    bd = np.zeros((NB, BS, NB, BS), np.float32)
    idx = np.arange(NB)
    bd[idx, :, idx, :] = np.transpose(np.asarray(w, np.float32), (0, 2, 1))
    return bd.reshape(INNER, INNER)


def _diag_tiles(bd):
    return np.ascontiguousarray(
        np.stack([bd[t * P:(t + 1) * P, t * P:(t + 1) * P] for t in range(NCH)])
    )


def _prep_direction(inp, d):
    """Fold weights for one direction into the per-core constant arrays."""
    g = lambda k: np.asarray(inp[f"{k}_{d}"], np.float32)
    bdq = _block_diag(g("Wq"))
    bdk = _block_diag(g("Wk"))
    bdv = _block_diag(g("Wv"))
    Wi, Wf = g("Wi"), g("Wf")
    # gates = cf @ (BDq Wi_q + BDk Wi_k) + xm @ (BDv Wi_v); i and f stacked
    wg_cf = np.concatenate(
        [bdq @ Wi[:INNER] + bdk @ Wi[INNER:2 * INNER],
         bdq @ Wf[:INNER] + bdk @ Wf[INNER:2 * INNER]], axis=1)  # (INNER, 8)
    wg_xm = np.concatenate(
        [bdv @ Wi[2 * INNER:], bdv @ Wf[2 * INNER:]], axis=1)     # (INNER, 8)
    scale = float(DH) ** -0.5
    return {
        "bdq": _diag_tiles(bdq * scale),
        "bdk": _diag_tiles(bdk),
        "bdv": _diag_tiles(bdv),
        "wg_cf": np.ascontiguousarray(wg_cf.reshape(NCH, P, 2 * NH)),
        "wg_xm": np.ascontiguousarray(wg_xm.reshape(NCH, P, 2 * NH)),
        "bi": np.asarray(inp[f"bi_{d}"], np.float32).reshape(NH, 1),
        "nbf": (-np.asarray(inp[f"bf_{d}"], np.float32)).reshape(NH, 1),
        "conv_w": np.asarray(inp[f"conv_w_{d}"], np.float32).reshape(NCH, P, KS),
        "conv_b": np.asarray(inp[f"conv_b_{d}"], np.float32).reshape(NCH, P, 1),
        "onw": np.asarray(inp[f"on_w_{d}"], np.float32).reshape(NCH, P, 1),
        "skip": np.asarray(inp[f"skip_{d}"], np.float32).reshape(NCH, P, 1),
    }


def _id4():
    a = np.zeros((P, NH), np.float32)
    for base in (0, 32, 64):
        a[base:base + NH, :] = np.eye(NH, dtype=np.float32)
    return a


def _shared_consts(inp):
    ln_w = np.asarray(inp["ln_w"], np.float32)
    ln_b = np.asarray(inp["ln_b"], np.float32)
    wup = np.asarray(inp["W_up"], np.float32)
    wup_eff = np.ascontiguousarray(ln_w[:, None] * wup)
    up_bias = (ln_b @ wup).astype(np.float32).reshape(2 * NCH, P, 1)
    jj = np.arange(P)[:, None]
    ii = np.arange(256)[None, :]
    maskA = (jj <= ii).astype(np.float32)          # J == 2p tile
    maskB = (jj + P <= ii).astype(np.float32)      # J == 2p+1 tile
    return {
        "wup": wup_eff,
        "upb": np.ascontiguousarray(up_bias),
        "wdown": np.asarray(inp["W_down"], np.float32),
        "maskA": np.ascontiguousarray(maskA),
        "id4": _id4(),
        "maskB": np.ascontiguousarray(maskB),
    }


# ----------------------------------------------------------------------------
# the Bass program (identical for all 8 cores)
# ----------------------------------------------------------------------------

def _build_program():
    nc = bacc.Bacc("TRN2", target_bir_lowering=False, debug=False,
                   num_devices=NCORES)

    dt = {}
    def din(name, shape):
        dt[name] = nc.dram_tensor(name, list(shape), F32,
                                  kind="ExternalInput").ap()
    din("x", (S, E))
    din("wup", (E, 2 * INNER))
    din("wdown", (INNER, E))
    din("upb", (2 * NCH, P, 1))
    din("maskA", (P, 256))
    din("id4", (P, NH))
    din("maskB", (P, 256))
    din("bdq", (NCH, P, P))
    din("bdk", (NCH, P, P))
    din("bdv", (NCH, P, P))
    din("wg_cf", (NCH, P, 2 * NH))
    din("wg_xm", (NCH, P, 2 * NH))
    din("bi", (NH, 1))
    din("nbf", (NH, 1))
    din("conv_w", (NCH, P, KS))
    din("conv_b", (NCH, P, 1))
    din("onw", (NCH, P, 1))
    din("skip", (NCH, P, 1))
    y_d = nc.dram_tensor("y", [S, E], F32, kind="ExternalOutput").ap()
    swz_d = nc.dram_tensor("swz_scratch", [INNER, S], F32).ap()

    with tile.TileContext(nc) as tc:
        _emit(nc, tc, dt, y_d, swz_d)
    nc.compile()
    return nc


def _emit(nc, tc, dt, y_d, swz_d):
    from contextlib import ExitStack

    with ExitStack() as top:
        consts = top.enter_context(tc.tile_pool(name="consts", bufs=1))
        work = top.enter_context(tc.tile_pool(name="work", bufs=2))
        mid = top.enter_context(tc.tile_pool(name="mid", bufs=1))
        late = top.enter_context(tc.tile_pool(name="late", bufs=1))

        ident = consts.tile([P, P], F32, tag="ident", name="ident")
        make_identity(nc, ident)
        maskA = consts.tile([P, 256], F32, tag="maskA", name="maskA")
        nc.gpsimd.dma_start(maskA, dt["maskA"])
        maskB = consts.tile([P, 256], F32, tag="maskB", name="maskB")
        nc.gpsimd.dma_start(maskB, dt["maskB"])
        id4 = consts.tile([P, NH], F32, tag="id4", name="id4")
        nc.gpsimd.dma_start(id4, dt["id4"])
        upb_t = []
        for mc in range(2 * NCH):
            t = consts.tile([P, 1], F32, tag=f"upb{mc}", name=f"upb{mc}")
            nc.gpsimd.dma_start(t, dt["upb"][mc])
            upb_t.append(t)
        bdv_t = []
        for cc in range(NCH):
            t = consts.tile([P, P], F32, tag=f"bdv{cc}", name=f"bdv{cc}")
            nc.gpsimd.dma_start(t, dt["bdv"][cc])
            bdv_t.append(t)
        wgcf_t, wgxm_t, cw_t, cb_t, onw_t, skip_t = [], [], [], [], [], []
        for cc in range(NCH):
            for lst, nm in ((wgcf_t, "wg_cf"), (wgxm_t, "wg_xm"),
                            (cw_t, "conv_w"), (cb_t, "conv_b"),
                            (onw_t, "onw"), (skip_t, "skip")):
                t = consts.tile(list(dt[nm].shape[1:]), F32,
                                tag=f"{nm}{cc}", name=f"{nm}{cc}")
                nc.gpsimd.dma_start(t, dt[nm][cc])
                lst.append(t)
        eps5 = consts.tile([P, 1], F32, tag="eps5", name="eps5")
        nc.vector.memset(eps5, 1e-5)
        bi_t = consts.tile([NH, 1], F32, tag="bi", name="bi")
        nc.gpsimd.dma_start(bi_t, dt["bi"])
        nbf_t = consts.tile([NH, 1], F32, tag="nbf", name="nbf")
        nc.gpsimd.dma_start(nbf_t, dt["nbf"])
        # transposed per-position gate quantities, filled in phase C1
        egT = consts.tile([P, NST * NH], F32, tag="egT", name="egT")
        emiT = consts.tile([P, NST * NH], F32, tag="emiT", name="emiT")
        eT = consts.tile([P, NST * NH], F32, tag="eT", name="eT")

        cfT = [mid.tile([P, S], F32, tag=f"cfT{c}", name=f"cfT{c}")
               for c in range(NCH)]
        hgnT = [late.tile([P, S], F32, tag=f"hgnT{c}", name=f"hgnT{c}")
                for c in range(NCH)]

        with tc.tile_pool(name="pBC", bufs=1) as pBC, \
             tc.tile_pool(name="psTr", bufs=2, space="PSUM") as psTr:
            vsb = [pBC.tile([P, NH, DH + 1], F32, tag=f"v{st}", name=f"v{st}")
                   for st in range(NST)]
            for st in range(NST):
                nc.vector.memset(vsb[st][:, :, DH:DH + 1], 1.0)

            with tc.tile_pool(name="psG", bufs=1, space="PSUM") as psG:
                pgi = psG.tile([NH, S], F32, tag="pgi", name="pgi")
                pgf = psG.tile([NH, S], F32, tag="pgf", name="pgf")

                # -------------------------------------------- phase A: LN
                with tc.tile_pool(name="pB", bufs=1) as pB, \
                     tc.tile_pool(name="psAB", bufs=2, space="PSUM") as psAB:
                    xnT = [pB.tile([P, S], F32, tag=f"xnT{e}", name=f"xnT{e}")
                           for e in range(NEC)]
                    for st in range(NST):
                        xt = work.tile([P, E], F32, tag="xt", name="xt")
                        nc.sync.dma_start(xt, dt["x"][st * P:(st + 1) * P, :])
                        st6 = work.tile([P, 6], F32, tag="st6", name="st6")
                        nc.vector.bn_stats(st6, xt)
                        mv = work.tile([P, 2], F32, tag="mv", name="mv")
                        nc.vector.bn_aggr(mv, st6)
                        sd = work.tile([P, 1], F32, tag="sd", name="sd")
                        nc.scalar.activation(sd, mv[:, 1:2], AF.Sqrt,
                                             bias=eps5[:, 0:1])
                        rs = work.tile([P, 1], F32, tag="rs", name="rs")
                        nc.vector.reciprocal(rs, sd)
                        xn = work.tile([P, E], F32, tag="xn", name="xn")
                        nc.vector.tensor_scalar(xn, xt, mv[:, 0:1], rs,
                                                ALU.subtract, ALU.mult)
                        for ec in range(NEC):
                            pt = psTr.tile([P, P], F32, tag="tr", name="tr")
                            nc.tensor.transpose(pt, xn[:, ec * P:(ec + 1) * P],
                                                ident)
                            nc.scalar.copy(xnT[ec][:, st * P:(st + 1) * P], pt)

                    # ------------------- phase B: up-proj, conv, v, gate accum
                    for quarter in range(4):
                        wupq = [pB.tile([P, 512], F32, tag=f"wupq{e}",
                                        name=f"wupq{e}", bufs=2)
                                for e in range(NEC)]
                        for ec in range(NEC):
                            nc.sync.dma_start(
                                wupq[ec],
                                dt["wup"][ec * P:(ec + 1) * P,
                                          quarter * 512:(quarter + 1) * 512])
                        for mq in range(4):
                            mc = quarter * 4 + mq
                            xmc = None
                            if mc < NCH:
                                xmc = pB.tile([P, S + 3], F32, tag="xmrot",
                                              name="xmrot", bufs=3)
                                nc.vector.memset(xmc[:, 0:1], 0.0)
                                nc.vector.memset(xmc[:, S + 1:S + 3], 0.0)
                            for nh2 in range(2):
                                ps = psAB.tile([P, 512], F32, tag="mm",
                                               name="mm")
                                for ec in range(NEC):
                                    nc.tensor.matmul(
                                        ps,
                                        lhsT=_r(wupq[ec][:, mq * P:(mq + 1) * P],
                                                R_UP),
                                        rhs=_r(xnT[ec][:, nh2 * 512:
                                                       (nh2 + 1) * 512], R_UP),
                                        start=(ec == 0), stop=(ec == NEC - 1))
                                if mc < NCH:
                                    nc.scalar.activation(
                                        xmc[:, 1 + nh2 * 512:
                                            1 + (nh2 + 1) * 512], ps,
                                        AF.Identity, bias=upb_t[mc][:, 0:1])
                                else:
                                    sg = work.tile([P, 512], F32, tag="sg",
                                                   name="sg")
                                    nc.scalar.activation(
                                        sg, ps, AF.Sigmoid,
                                        bias=upb_t[mc][:, 0:1])
                                    szt = work.tile([P, 512], F32, tag="szt",
                                                    name="szt")
                                    nc.vector.scalar_tensor_tensor(
                                        szt, ps, upb_t[mc][:, 0:1], sg,
                                        ALU.add, ALU.mult)
                                    nc.sync.dma_start(
                                        swz_d[(mc - NCH) * P:(mc - NCH + 1) * P,
                                              nh2 * 512:(nh2 + 1) * 512], szt)
                            if mc >= NCH:
                                continue
                            cc = mc
                            # depthwise conv (4 taps) + bias + swish -> cfT[cc]
                            y0 = work.tile([P, S], F32, tag="cva", name="cva",
                                           bufs=1)
                            nc.vector.tensor_scalar(y0, xmc[:, 0:S],
                                                    cw_t[cc][:, 0:1], None,
                                                    ALU.mult)
                            y1 = work.tile([P, S], F32, tag="cvb", name="cvb",
                                           bufs=1)
                            nc.vector.scalar_tensor_tensor(
                                y1, xmc[:, 1:1 + S], cw_t[cc][:, 1:2], y0,
                                ALU.mult, ALU.add)
                            y2 = work.tile([P, S], F32, tag="cva", name="cva",
                                           bufs=1)
                            nc.vector.scalar_tensor_tensor(
                                y2, xmc[:, 2:2 + S], cw_t[cc][:, 2:3], y1,
                                ALU.mult, ALU.add)
                            y3 = work.tile([P, S], F32, tag="cvb", name="cvb",
                                           bufs=1)
                            nc.vector.scalar_tensor_tensor(
                                y3, xmc[:, 3:3 + S], cw_t[cc][:, 3:4], y2,
                                ALU.mult, ALU.add)
                            sgc = work.tile([P, S], F32, tag="cvs", name="cvs",
                                            bufs=1)
                            nc.scalar.activation(sgc, y3, AF.Sigmoid,
                                                 bias=cb_t[cc][:, 0:1])
                            nc.vector.scalar_tensor_tensor(
                                cfT[cc], y3, cb_t[cc][:, 0:1], sgc,
                                ALU.add, ALU.mult)
                            # v (S-major, with trailing ones column)
                            for st in range(NST):
                                vp = psTr.tile([P, P], F32, tag="tr", name="tr")
                                nc.tensor.matmul(
                                    vp,
                                    lhsT=_r(xmc[:, 1 + st * P:1 + (st + 1) * P],
                                            R_PROJ),
                                    rhs=_r(bdv_t[cc], R_PROJ),
                                    start=True, stop=True)
                                nc.scalar.copy(
                                    vsb[st][:, cc // 2,
                                            (cc % 2) * P:(cc % 2) * P + P], vp)
                            # gate projections, accumulated across chunks
                            for half in range(2):
                                hsl = slice(half * 512, (half + 1) * 512)
                                for pg, c0 in ((pgi, 0), (pgf, NH)):
                                    nc.tensor.matmul(
                                        pg[:, hsl],
                                        lhsT=_r(wgxm_t[cc][:, c0:c0 + NH],
                                                R_PROJ),
                                        rhs=_r(xmc[:, 1 + half * 512:
                                                   1 + (half + 1) * 512],
                                               R_PROJ),
                                        start=(cc == 0), stop=False)
                                    nc.tensor.matmul(
                                        pg[:, hsl],
                                        lhsT=_r(wgcf_t[cc][:, c0:c0 + NH],
                                                R_PROJ),
                                        rhs=_r(cfT[cc][:, hsl], R_PROJ),
                                        start=False, stop=(cc == NCH - 1))

                # --------------------- phase C1: gate post-processing (O(S))
                with tc.tile_pool(name="pGate", bufs=1) as pGate:
                    gq = {}
                    for nm in ("ipre", "sp", "csp", "g", "M", "eg", "emi",
                               "efi"):
                        gq[nm] = pGate.tile([NH, S], F32, tag=nm, name=nm)
                    ipre, sp_r, csp, g_r = (gq["ipre"], gq["sp"], gq["csp"],
                                            gq["g"])
                    M_r, eg_r, emi_r, efi = (gq["M"], gq["eg"], gq["emi"],
                                             gq["efi"])
                    nc.scalar.activation(ipre, pgi, AF.Identity,
                                         bias=bi_t[:, 0:1])
                    ut = work.tile([NH, S], F32, tag="ut", name="ut")
                    nc.scalar.activation(ut, pgf, AF.Exp,
                                         bias=nbf_t[:, 0:1], scale=-1.0)
                    nc.scalar.activation(sp_r, ut, AF.Ln, bias=1.0)
                    nc.vector.tensor_tensor_scan(csp, sp_r, sp_r, 0.0,
                                                 ALU.add, ALU.bypass)
                    nc.vector.tensor_tensor(g_r, ipre, csp, ALU.add)
                    nc.vector.tensor_tensor_scan(M_r, g_r, g_r, -3.0e38,
                                                 ALU.max, ALU.bypass)
                    nc.scalar.activation(eg_r, g_r, AF.Exp)
                    nc.scalar.activation(emi_r, M_r, AF.Exp, scale=-1.0)
                    nc.vector.tensor_tensor(efi, csp, M_r, ALU.subtract)
                    efl = ipre  # tile reuse: ipre no longer needed
                    nc.scalar.activation(efl, efi, AF.Exp)
                    for st in range(NST):
                        for srcq, dstT in ((eg_r, egT), (emi_r, emiT),
                                           (efl, eT)):
                            pt = psTr.tile([P, NH], F32, tag="tr", name="tr")
                            nc.tensor.transpose(pt,
                                                srcq[:, st * P:(st + 1) * P],
                                                id4[0:NH, :])
                            nc.scalar.copy(dstT[:, st * NH:(st + 1) * NH], pt)

            # --------------------------------- phase C2: mLSTM attention part
            with tc.tile_pool(name="pC2", bufs=1) as pC2, \
                 tc.tile_pool(name="psQK", bufs=2, space="PSUM") as psQK, \
                 tc.tile_pool(name="psH", bufs=2, space="PSUM") as psH, \
                 tc.tile_pool(name="psPr", bufs=2, space="PSUM") as psPr:
                bdq_t, bdk_t = [], []
                for nm, lst in (("bdq", bdq_t), ("bdk", bdk_t)):
                    for cc in range(NCH):
                        t = pC2.tile([P, P], F32, tag=f"{nm}{cc}",
                                     name=f"{nm}{cc}")
                        nc.gpsimd.dma_start(t, dt[nm][cc])
                        lst.append(t)
                for h in range(NH):
                    ccs = (2 * h, 2 * h + 1)
                    qT, kT = [], []
                    for nm, bdt, lst in (("q", bdq_t, qT), ("k", bdk_t, kT)):
                        for ci, cc in enumerate(ccs):
                            t = pC2.tile([P, S], F32, tag=f"{nm}T{ci}",
                                         name=f"{nm}T{ci}")
                            for nh2 in range(2):
                                ps = psPr.tile([P, 512], F32, tag="proj",
                                               name="proj")
                                nc.tensor.matmul(
                                    ps, lhsT=_r(bdt[cc], R_PROJ),
                                    rhs=_r(cfT[cc][:, nh2 * 512:
                                                   (nh2 + 1) * 512], R_PROJ),
                                    start=True, stop=True)
                                nc.scalar.copy(t[:, nh2 * 512:(nh2 + 1) * 512],
                                               ps)
                            lst.append(t)
                    for pp in range(4):  # i-block pairs (i span of 256)
                        nJ = 2 * pp + 2
                        hps = [psH.tile([P, DH + 1], F32, tag="h", name="h")
                               for _ in range(2)]
                        for jt in range(nJ):
                            qk = psQK.tile([P, 256], F32, tag="qk", name="qk")
                            for ci in range(2):
                                nc.tensor.matmul(
                                    qk,
                                    lhsT=_r(kT[ci][:, jt * P:(jt + 1) * P],
                                            R_ATT),
                                    rhs=_r(qT[ci][:, pp * 256:(pp + 1) * 256],
                                           R_ATT),
                                    start=(ci == 0), stop=(ci == 1))
                            ct = work.tile([P, 256], F32, tag="ct", name="ct",
                                           bufs=3)
                            egc = egT[:, jt * NH + h:jt * NH + h + 1]
                            if jt == 2 * pp:
                                nc.vector.scalar_tensor_tensor(
                                    ct, qk, egc, maskA, ALU.mult, ALU.mult)
                            elif jt == 2 * pp + 1:
                                nc.vector.scalar_tensor_tensor(
                                    ct, qk, egc, maskB, ALU.mult, ALU.mult)
                            else:
                                nc.vector.tensor_scalar(ct, qk, egc, None,
                                                        ALU.mult)
                            for ih in range(2):
                                nc.tensor.matmul(
                                    hps[ih],
                                    lhsT=_r(ct[:, ih * P:(ih + 1) * P], R_ATT),
                                    rhs=_r(vsb[jt][:, h, :], R_ATT),
                                    start=(jt == 0), stop=(jt == nJ - 1))
                        for ih in range(2):
                            bi_idx = 2 * pp + ih
                            col = bi_idx * NH + h
                            emic = emiT[:, col:col + 1]
                            efc = eT[:, col:col + 1]
                            t1 = work.tile([P, 1], F32, tag="n1", name="n1")
                            nc.scalar.activation(t1, hps[ih][:, DH:DH + 1],
                                                 AF.Abs, scale=emic)
                            t2 = work.tile([P, 1], F32, tag="n2", name="n2")
                            nc.vector.tensor_scalar(
                                t2, t1, efc, 1e-6, ALU.max,
                                ALU.add)
                            t3 = work.tile([P, 1], F32, tag="n3", name="n3")
                            nc.vector.reciprocal(t3, t2)
                            sc = work.tile([P, 1], F32, tag="n4", name="n4")
                            nc.vector.tensor_scalar(sc, t3, emic,
                                                    None, ALU.mult)
                            hs = work.tile([P, DH], F32, tag="hs", name="hs")
                            nc.vector.tensor_scalar(hs, hps[ih][:, 0:DH], sc,
                                                    None, ALU.mult)
                            st6 = work.tile([P, 6], F32, tag="g6", name="g6")
                            nc.vector.bn_stats(st6, hs)
                            mv = work.tile([P, 2], F32, tag="gmv", name="gmv")
                            nc.vector.bn_aggr(mv, st6)
                            sd = work.tile([P, 1], F32, tag="gsd", name="gsd")
                            nc.scalar.activation(sd, mv[:, 1:2], AF.Sqrt,
                                                 bias=eps5[:, 0:1])
                            rsd = work.tile([P, 1], F32, tag="grs", name="grs")
                            nc.vector.reciprocal(rsd, sd)
                            hn = work.tile([P, DH], F32, tag="hn", name="hn")
                            nc.vector.tensor_scalar(hn, hs, mv[:, 0:1], rsd,
                                                    ALU.subtract, ALU.mult)
                            for ch in range(2):
                                pt = psTr.tile([P, P], F32, tag="tr", name="tr")
                                nc.tensor.transpose(
                                    pt, hn[:, ch * P:(ch + 1) * P], ident)
                                nc.scalar.copy(
                                    hgnT[2 * h + ch][:,
                                                     bi_idx * P:(bi_idx + 1) * P],
                                    pt)

        # ------------------------------------------------- phase D: down-proj
        with tc.tile_pool(name="pD", bufs=1) as pD, \
             tc.tile_pool(name="psD", bufs=3, space="PSUM") as psD:
            wdT = [pD.tile([P, E], F32, tag=f"wd{c}", name=f"wd{c}")
                   for c in range(NCH)]
            for cc in range(NCH):
                nc.sync.dma_start(wdT[cc], dt["wdown"][cc * P:(cc + 1) * P, :])
            for cc in range(NCH):
                swt = pD.tile([P, S], F32, tag="swt", name="swt", bufs=2)
                nc.sync.dma_start(swt, swz_d[cc * P:(cc + 1) * P, :])
                t1 = pD.tile([P, S], F32, tag="d1", name="d1", bufs=1)
                nc.vector.tensor_scalar(t1, cfT[cc], skip_t[cc][:, 0:1],
                                        None, ALU.mult)
                t2 = pD.tile([P, S], F32, tag="d2", name="d2", bufs=1)
                nc.vector.scalar_tensor_tensor(
                    t2, hgnT[cc], onw_t[cc][:, 0:1], t1, ALU.mult, ALU.add)
                nc.vector.tensor_tensor(hgnT[cc], t2, swt, ALU.mult)
            for st in range(NST):
                ps = psD.tile([P, E], F32, tag="dmm", name="dmm")
                for cc in range(NCH):
                    nc.tensor.matmul(
                        ps, lhsT=_r(hgnT[cc][:, st * P:(st + 1) * P], R_DOWN),
                        rhs=_r(wdT[cc], R_DOWN),
                        start=(cc == 0), stop=(cc == NCH - 1))
                ot = pD.tile([P, E], F32, tag="ot", name="ot", bufs=2)
                nc.scalar.copy(ot, ps)
                nc.sync.dma_start(y_d[st * P:(st + 1) * P, :], ot)


# ----------------------------------------------------------------------------
# host entry points
# ----------------------------------------------------------------------------

_PROG = None


def _get_program():
    global _PROG
    if _PROG is None:
        _PROG = _build_program()
    return _PROG


def _make_in_maps(inputs):
    x = np.asarray(inputs["x"], np.float32)
    shared = _shared_consts(inputs)
    per_dir = {d: _prep_direction(inputs, d) for d in ("f", "r")}
    in_maps = []
    for b in range(B):
        for d in ("f", "r"):
            xb = x[b] if d == "f" else x[b, ::-1]
            m = {"x": np.ascontiguousarray(xb)}
            m.update(shared)
            m.update(per_dir[d])
            in_maps.append(m)
    return x, in_maps


def _combine(x, results):
    out = np.empty((B, S, E), np.float32)
    for b in range(B):
        yf = results[2 * b]["y"]
        yr = results[2 * b + 1]["y"]
        out[b] = x[b] + yf + yr[::-1]
    return out


def kernel(**inputs):
    from concourse.bass_utils import run_bass_kernel_spmd
    nc = _get_program()
    x, in_maps = _make_in_maps(inputs)
    res = run_bass_kernel_spmd(nc, in_maps, core_ids=list(range(NCORES)))
    return _combine(x, res.results)


def run_traced(inputs, **kw):
    """Returns (out, BassKernelResults) with profiling enabled."""
    from concourse.bass_utils import run_bass_kernel_spmd
    nc = _get_program()
    x, in_maps = _make_in_maps(inputs)
    res = run_bass_kernel_spmd(nc, in_maps, core_ids=list(range(NCORES)),
                               trace=True, **kw)
    return _combine(x, res.results), res


def run_sim(inputs, core=0):
    """Simulate one core's program in CoreSim; returns that core's y."""
    from concourse.bass_interp import CoreSim
    nc = _get_program()
    x, in_maps = _make_in_maps(inputs)
    sim = CoreSim(nc)
    for k, v in in_maps[core].items():
        sim.tensor(k)[:] = v
    sim.simulate()
    return np.array(sim.tensor("y")), x
